# revision 8
# baseline (speedup 1.0000x reference)
"""Binarized linear block (y = relu(batchnorm(x @ sign(W).T))) on 8 TRN2 NeuronCores.

Strategy (v3 — 2D sharding + remote-DMA sync-BN):
  - 2D shard: batch 4-way x output-channels 2-way.  Core c handles batch
    shard c%4 (4096 rows) and channel half c//4 (8 of 16 channel tiles).
    Sync-BN spans only the 4 same-die cores sharing a channel half.
  - The cross-core exchange of per-channel second moments uses direct
    SBUF->SBUF remote DMA broadcasts (XOR-relative quad addressing)
    instead of a CC AllGather: ~2us one-hop latency vs ~7-9us for the
    4-rank ncfw mesh (and ~15-19us for 8-rank).  Receivers wait on
    per-sender semaphores (FIFO per link), so the scheme is race-free
    under arbitrary core skew.  The waits are emitted as trivially-true
    placeholders (the Tile scheduling sim cannot model remote
    increments) and the real thresholds are patched in post-schedule.
  - Weights binarized + transposed + tiled on host.  Mixed precision on
    the contraction: the first KBN_N8 k-tiles (of 16) run as fp8e4m3
    DoubleRow matmuls (2 k-tiles per PE pass), the rest as fp16.
    sign(W) is exact in both dtypes; only x's fp8 rounding costs
    accuracy (~2.6% elementwise on the fp8 fraction).
  - x is pre-transposed so the contraction dim lies on SBUF partitions;
    matmuls compute y^T tiles [o x b] so BN batch statistics are
    per-partition free-dim reductions.
  - The BN batch MEAN is computed exactly on the host from the quantized
    x (colsum @ sign(W) is cheap) and shipped as an input; only the
    per-channel second moment needs the cross-core exchange.
  - Channel groups [2,2,2,1,1]: the last two groups are single-tile so
    the exposed tail after the final exchange is one lean finish
    (stats math + one [128,4096] normalize + 1MB store).
  - Output is written as y^T tiles [m, 128, b_loc] in fp16 (halves the
    store traffic; BN output is ~unit-scale so fp16 adds ~0.05% error);
    host transposes back and upcasts to fp32.
"""

import os as _os

import numpy as np

_BN_EPS = 1e-5

# number of k-tiles (of in_dim/128) computed via fp8 DoubleRow; even.
_N8 = int(_os.environ.get("KBN_N8", "8"))

# batch shards x channel shards (must multiply to n_cores)
_DP = 4
_CP = 2

_CACHE = {}


def _env(name):
    return bool(_os.environ.get(name))


def _group_sizes(mt):
    # Pairs first, two single-tile groups last: the final exposed
    # exchange covers one channel tile and its finish path is lean.
    if mt <= 2:
        return [1] * mt
    if mt % 2:
        return [2] * ((mt - 3) // 2) + [1, 1, 1]
    return [2] * ((mt - 2) // 2) + [1, 1]


def _build(n_cores, b_total, in_dim, out_dim):
    import concourse.bass as bass  # noqa: F401
    import concourse.mybir as mybir
    import concourse.tile as tile
    from concourse import bacc

    f16 = mybir.dt.bfloat16 if _env("KBN_BF16") else mybir.dt.float16
    f32 = mybir.dt.float32
    f8 = mybir.dt.float8e4
    AF = mybir.ActivationFunctionType
    PM = mybir.MatmulPerfMode

    W = _DP                   # sync-BN world (batch shards)
    b_loc = b_total // _DP    # rows per core
    MT = out_dim // 128 // _CP  # local output-channel tiles
    KT = in_dim // 128        # k tiles (contraction)
    CH = min(512, b_loc)      # moving-operand chunk
    NCH = b_loc // CH         # batch chunks
    n8 = max(0, min(_N8, KT)) // 2 * 2
    T8 = n8 // 2              # fp8 DoubleRow k-tile pairs
    K16 = KT - n8             # fp16 k tiles
    groups = _group_sizes(MT)

    nc = bacc.Bacc(
        "TRN2",
        target_bir_lowering=False,
        debug=False,
        enable_asserts=False,
        num_devices=n_cores,
    )

    # layouts put the contraction dim on partitions; batch-chunk DMAs are
    # contiguous per partition
    xt = wt = x8 = w8 = None
    if K16:
        xt = nc.dram_tensor("xt", [128, NCH, K16, CH], f16, kind="ExternalInput")
        wt = nc.dram_tensor("wt", [MT, 128, K16, 128], f16, kind="ExternalInput")
    if T8:
        x8 = nc.dram_tensor("x8", [128, NCH, T8, 2, CH], f8, kind="ExternalInput")
        w8 = nc.dram_tensor("w8", [MT, 128, T8, 2, 128], f8, kind="ExternalInput")
    gmt = nc.dram_tensor("gmt", [128, MT], f32, kind="ExternalInput")
    bta = nc.dram_tensor("bta", [128, MT], f32, kind="ExternalInput")
    # exact batch mean (host-computed) and its square, in [128, MT] layout
    mut = nc.dram_tensor("mut", [128, MT], f32, kind="ExternalInput")
    mu2 = nc.dram_tensor("mu2", [128, MT], f32, kind="ExternalInput")
    out = nc.dram_tensor("out", [MT, 128, b_loc], f16, kind="ExternalOutput")

    # patched post-schedule: (wait_inst, sem, threshold)
    patch_waits = []

    with tile.TileContext(nc) as tc:
        with (
            tc.tile_pool(name="xpool", bufs=1) as xpool,
            tc.tile_pool(name="wpool", bufs=3) as wpool,
            tc.tile_pool(name="ypool", bufs=MT) as ypool,
            tc.tile_pool(name="opool", bufs=4) as opool,
            tc.tile_pool(name="stat", bufs=1) as stat,
            tc.tile_pool(name="gstat", bufs=4) as gstat,
            tc.tile_pool(name="psum", bufs=7, space="PSUM") as psum,
        ):
            xt_sb = x8_sb = None
            if K16:
                xt_sb = xpool.tile([128, NCH, K16, CH], f16, name="xt_sb")
            if T8:
                x8_sb = xpool.tile([128, NCH, T8, 2, CH], f8, name="x8_sb")

            # per-sender-direction arrival semaphores: rsems[k-1] counts
            # data from quad peer (self ^ k); +2 per group send
            rsems = [nc.alloc_semaphore(f"rsem{k}") for k in (1, 2, 3)]
            lsem = nc.alloc_semaphore("lsem")

            # Startup: k-sliced loads so the first matmuls are gated on
            # <1MB, interleaved fp8/fp16 per chunk so chunk 0's fp16
            # passes aren't queued behind every fp8 chunk.
            wts = []
            w8_0 = w16_0 = w8_1 = w16_1 = None
            if T8:
                w8_0 = wpool.tile([128, T8, 2, 128], f8, tag="w8")
                nc.sync.dma_start(w8_0[:], w8.ap()[0])
                th = max(T8 // 2, 1)
                nc.sync.dma_start(x8_sb[:, 0, :th], x8.ap()[:, 0, :th])
                nc.sync.dma_start(x8_sb[:, 0, th:], x8.ap()[:, 0, th:])
                if MT > 1:
                    w8_1 = wpool.tile([128, T8, 2, 128], f8, tag="w8")
                    nc.sync.dma_start(w8_1[:], w8.ap()[1])
            if K16:
                w16_0 = wpool.tile([128, K16, 128], f16, tag="wt")
                kh = max(K16 // 2, 1)
                kq = max(K16 // 4, 1)
                nc.sync.dma_start(w16_0[:, :kh], wt.ap()[0, :, :kh])
                nc.sync.dma_start(xt_sb[:, 0, :kq], xt.ap()[:, 0, :kq])
                nc.sync.dma_start(xt_sb[:, 0, kq:kh], xt.ap()[:, 0, kq:kh])
                nc.sync.dma_start(w16_0[:, kh:], wt.ap()[0, :, kh:])
                nc.sync.dma_start(xt_sb[:, 0, kh : kh + kq], xt.ap()[:, 0, kh : kh + kq])
                nc.sync.dma_start(xt_sb[:, 0, kh + kq :], xt.ap()[:, 0, kh + kq :])
                if MT > 1:
                    w16_1 = wpool.tile([128, K16, 128], f16, tag="wt")
                    nc.sync.dma_start(w16_1[:], wt.ap()[1])
            # remaining chunks interleaved: consumption order is per-chunk
            for n in range(1, NCH):
                if T8:
                    nc.sync.dma_start(x8_sb[:, n], x8.ap()[:, n])
                if K16:
                    nc.sync.dma_start(xt_sb[:, n], xt.ap()[:, n])
            wts.append((w8_0, w16_0))
            if MT > 1:
                wts.append((w8_1, w16_1))

            gamma_sb = stat.tile([128, MT], f32)
            beta_sb = stat.tile([128, MT], f32)
            mu_sb = stat.tile([128, MT], f32)
            mu2_sb = stat.tile([128, MT], f32)
            # NOTE: must NOT use gpsimd.dma_start here -- mainline SWDGE is
            # pinned to queue 0, the same ring the remote_dma descriptors
            # ride; mixing them corrupts the trigger accounting.
            nc.scalar.dma_start(gamma_sb[:], gmt.ap())
            nc.scalar.dma_start(beta_sb[:], bta.ap())
            nc.scalar.dma_start(mu_sb[:], mut.ap())
            nc.scalar.dma_start(mu2_sb[:], mu2.ap())

            # nb = eps - mu^2: folds the variance computation into the
            # Sqrt activation's bias
            nb_t = stat.tile([128, MT], f32)
            nc.gpsimd.memset(nb_t[:], _BN_EPS)
            nc.gpsimd.tensor_sub(nb_t[:], nb_t[:], mu2_sb[:])

            yts = [None] * MT
            last_mm = [None]    # most recent matmul instruction
            last_act = [None]   # most recent PSUM-drain ACTIVATE (ScalarE)
            last_stat = [None]  # most recent bn_stats (VectorE)

            def pin(inst, anchor):
                # order-only (no semaphore) same-engine pin: keeps
                # exchange-dependent ops from being scheduled ahead of
                # stream work on the strict-FIFO engine queues
                if anchor[0] is not None:
                    tile.add_dep_helper(
                        inst.ins,
                        anchor[0].ins,
                        sync=False,
                        reason="pin exchange-dependent op behind stream",
                    )

            def emit_dr(wpair, n):
                """The fp8 DoubleRow passes of one batch-chunk (opens PSUM)."""
                ps = psum.tile([128, CH], f32, name="ps")
                w8_m = wpair[0]
                for t in range(T8):
                    last_mm[0] = nc.tensor.matmul(
                        ps[:],
                        w8_m[:, t],
                        x8_sb[:, n, t],
                        start=(t == 0),
                        stop=(K16 == 0 and t == T8 - 1),
                        perf_mode=PM.DoubleRow,
                    )
                return ps

            def emit_f16(m, wpair, bns, j, n, ps):
                """fp16 passes + epilogues of one batch-chunk (closes PSUM)."""
                w16_m = wpair[1]
                ns = slice(n * CH, (n + 1) * CH)
                for k in range(K16):
                    last_mm[0] = nc.tensor.matmul(
                        ps[:],
                        w16_m[:, k, :],
                        xt_sb[:, n, k, :],
                        start=(T8 == 0 and k == 0),
                        stop=(k == K16 - 1),
                    )
                # VectorE: batch stats first (PSUM bank is single-port, the
                # two readers serialize -- stats lead the exchange chain);
                # then ScalarE: fp16 copy of y^T
                last_stat[0] = nc.vector.bn_stats(out=bns[:, j, n, :], in_=ps[:])
                last_act[0] = nc.scalar.activation(yts[m][:, ns], ps[:], AF.Identity)

            def emit_chunk(m, wpair, bns, j, n):
                """One (channel-tile, batch-chunk): matmuls + epilogues."""
                ps = emit_dr(wpair, n)
                emit_f16(m, wpair, bns, j, n, ps)

            def emit_collective(m0, gm, bns, gi):
                """Pack the group's second moments, broadcast to quad peers."""
                # local (mean, var) per channel tile in the group
                mv = gstat.tile([128, gm, 2], f32, tag=f"mv{gi}")
                for j in range(gm):
                    ag = nc.vector.bn_aggr(out=mv[:, j, :], in_=bns[:, j])
                    pin(ag, last_stat)
                    last_stat[0] = ag

                # per-core second moment m2 = var + mean^2
                st = gstat.tile([128, gm], f32, tag=f"st{gi}")
                nc.vector.tensor_mul(st[:], mv[:, :, 0], mv[:, :, 0])
                # chain last_stat through st so every later arrival-wait is
                # pinned AFTER this group's exchange payload is produced --
                # otherwise the scheduler may park a blocking wait ahead of
                # it on the strict-FIFO vector queue (cross-core deadlock)
                last_stat[0] = nc.vector.tensor_add(st[:], mv[:, :, 1], st[:])

                # direct SBUF->SBUF sends to the 3 XOR-quad peers; call k
                # lands peer (self^k)'s data in gbuf[:, k-1, :] (sender-
                # permuted per receiver, but we only sum the columns)
                gbuf = gstat.tile([128, 3, gm], f32, tag=f"gbuf{gi}")
                if _env("KBN_NO_RDMA"):
                    for k in (1, 2, 3):
                        nc.vector.tensor_copy(gbuf[:, k - 1, :], st[:])
                else:
                    for k in (1, 2, 3):
                        rd = [None] * 8
                        rd[k] = (0, k)
                        nc.gpsimd.remote_dma_broadcast(
                            gbuf[:, k - 1, :], st[:], rsems[k - 1], lsem, rdests=rd
                        )
                    nc.gpsimd.trigger_dma(3)
                return (m0, gm, gbuf, st, gi)

            def emit_finish(state, split=4):
                m0, gm, gbuf, st, gi = state
                # arrival waits: peer (self^k)'s group-gi send gives +2 on
                # rsems[k-1]; placeholder 0-waits here, real thresholds
                # patched post-schedule (the scheduling sim cannot model
                # remote increments)
                if not _env("KBN_NO_RDMA"):
                    for k in (1, 2, 3):
                        wv = nc.vector.wait_ge(rsems[k - 1], 0)
                        pin(wv, last_stat)
                        patch_waits.append((wv, rsems[k - 1], 2 * (gi + 1)))
                        last_stat[0] = wv

                s4 = gstat.tile([128, gm], f32, tag=f"s4{gi}")
                a1 = nc.vector.tensor_add(s4[:], st[:], gbuf[:, 0, :])
                pin(a1, last_stat)
                last_stat[0] = a1
                nc.vector.tensor_add(s4[:], s4[:], gbuf[:, 1, :])
                last_stat[0] = nc.vector.tensor_add(s4[:], s4[:], gbuf[:, 2, :])

                ms = slice(m0, m0 + gm)
                sd_t = gstat.tile([128, gm], f32, tag="sd")
                inv_t = gstat.tile([128, gm], f32, tag="inv")
                scale_t = gstat.tile([128, gm], f32, tag="scale")
                tmp_t = gstat.tile([128, gm], f32, tag="tmp")
                shift_t = gstat.tile([128, gm], f32, tag="shift")
                # lean path: sd = sqrt(m2_sum/W + (eps - mu^2)), variance
                # folded into the per-column activation bias
                for j in range(gm):
                    sq = nc.scalar.activation(
                        sd_t[:, j : j + 1],
                        s4[:, j : j + 1],
                        AF.Sqrt,
                        bias=nb_t[:, m0 + j : m0 + j + 1],
                        scale=1.0 / W,
                    )
                    pin(sq, last_act)
                rc = nc.vector.reciprocal(inv_t[:], sd_t[:])
                pin(rc, last_stat)
                sc = nc.vector.tensor_mul(scale_t[:], gamma_sb[:, ms], inv_t[:])
                pin(sc, last_stat)
                tm = nc.vector.tensor_mul(tmp_t[:], mu_sb[:, ms], scale_t[:])
                pin(tm, last_stat)
                sh = nc.vector.tensor_sub(shift_t[:], beta_sb[:, ms], tmp_t[:])
                pin(sh, last_stat)

                hb = b_loc // split
                for j, m in enumerate(range(m0, m0 + gm)):
                    for h in range(split):
                        hs = slice(h * hb, (h + 1) * hb)
                        out_h = opool.tile([128, hb], f16)
                        # split normalize across ScalarE and DVE so the
                        # serial tail halves
                        use_dve = split > 1 and (
                            (gm == 2 and j == 0) or (gm == 1 and h % 2 == 1)
                        )
                        if use_dve:
                            v1 = nc.vector.tensor_scalar(
                                out_h[:],
                                yts[m][:, hs],
                                scale_t[:, j : j + 1],
                                shift_t[:, j : j + 1],
                                mybir.AluOpType.mult,
                                mybir.AluOpType.add,
                            )
                            pin(v1, last_stat)
                            v2 = nc.vector.tensor_scalar_max(
                                out_h[:], out_h[:], 0.0
                            )
                            pin(v2, last_stat)
                        else:
                            act = nc.scalar.activation(
                                out_h[:],
                                yts[m][:, hs],
                                AF.Relu,
                                bias=shift_t[:, j : j + 1],
                                scale=scale_t[:, j : j + 1],
                            )
                            pin(act, last_act)
                        nc.sync.dma_start(out.ap()[m, :, hs], out_h[:])

            # Pipeline: batch-chunk-outer within each group. Group g's
            # exchange is emitted after group g+1's first chunk pass; its
            # finish one exchange later (the ~2us hop latency is well
            # under the group spacing). The last two groups are single-
            # tile: the final exposed exchange covers one channel tile
            # with a lean finish critical path.
            states = []
            pend_coll = None
            m0 = 0
            gi = 0
            G = len(groups)
            for g, gm in enumerate(groups):
                ms = list(range(m0, m0 + gm))
                bns = gstat.tile([128, gm, NCH, 6], f32, tag="bns")
                wtiles = []
                for m in ms:
                    if m < len(wts):
                        wtiles.append(wts[m])
                    else:
                        w8_m = w16_m = None
                        if T8:
                            w8_m = wpool.tile([128, T8, 2, 128], f8, tag="w8")
                            nc.sync.dma_start(w8_m[:], w8.ap()[m])
                        if K16:
                            w16_m = wpool.tile([128, K16, 128], f16, tag="wt")
                            nc.sync.dma_start(w16_m[:], wt.ap()[m])
                        wtiles.append((w8_m, w16_m))
                    yt_m = ypool.tile([128, b_loc], f16, tag="yt")
                    yts[m] = yt_m
                for n in range(NCH):
                    if g == 0 and n == 0 and gm > 1 and T8 and K16:
                        # cold start: both tiles' DoubleRow passes first --
                        # they need only the small fp8 inputs, buying time
                        # for chunk 0's fp16 x slices to land
                        pss = [emit_dr(wtiles[j], n) for j in range(gm)]
                        for j, m in enumerate(ms):
                            emit_f16(m, wtiles[j], bns, j, n, pss[j])
                    else:
                        for j, m in enumerate(ms):
                            emit_chunk(m, wtiles[j], bns, j, n)
                    if n == 0 and pend_coll is not None:
                        states.append(emit_collective(*pend_coll, gi))
                        gi += 1
                        pend_coll = None
                        if len(states) > 1:
                            emit_finish(states.pop(0))
                if g == G - 1:
                    states.append(emit_collective(m0, gm, bns, gi))
                    gi += 1
                    if len(states) > 2:
                        emit_finish(states.pop(0))
                else:
                    pend_coll = (m0, gm, bns)
                m0 += gm
            if pend_coll is not None:
                states.append(emit_collective(*pend_coll, gi))
                gi += 1
            for state in states:
                emit_finish(state)

    # patch the real arrival thresholds into the placeholder waits (the
    # scheduling sim has already run; hardware honors the added condition)
    for wv, sem, thr in patch_waits:
        wv.wait_op(sem, thr, "sem-ge")

    nc.compile()
    return nc


def _get_nc(key):
    if key not in _CACHE:
        _CACHE[key] = _build(*key)
    return _CACHE[key]


def _prepare_in_maps(x, weight, gamma, beta, n_cores):
    import ml_dtypes

    b_total, in_dim = x.shape
    out_dim = weight.shape[0]
    b_loc = b_total // _DP
    KT = in_dim // 128
    MT = out_dim // 128 // _CP
    CH = min(512, b_loc)
    NCH = b_loc // CH
    n8 = max(0, min(_N8, KT)) // 2 * 2
    T8 = n8 // 2
    K16 = KT - n8
    split = n8 * 128

    _hdt = ml_dtypes.bfloat16 if _env("KBN_BF16") else np.float16
    f8np = ml_dtypes.float8_e4m3

    # host-side marshalling (binarize / transpose / cast / tile)
    wb = np.where(weight >= 0, np.float32(1.0), np.float32(-1.0))
    if n8:
        x8h = x[:, :split].astype(f8np)
    if K16:
        x16h = x[:, split:].astype(_hdt)

    # exact batch mean of y (built from the quantized x the kernel
    # actually uses): mean = colsum(x_quant) @ sign(W)^T / B
    colsum = np.zeros(in_dim, dtype=np.float64)
    if n8:
        colsum[:split] = x8h.astype(np.float64).sum(axis=0)
    if K16:
        colsum[split:] = x16h.astype(np.float64).sum(axis=0)
    mu = (wb.astype(np.float64) @ colsum) / b_total  # [out]

    # per-channel-shard constant tensors
    cmaps = []
    for cs in range(_CP):
        os_ = slice(cs * MT * 128, (cs + 1) * MT * 128)
        cm = {
            "gmt": np.ascontiguousarray(
                gamma[os_].reshape(MT, 128).T.astype(np.float32)
            ),
            "bta": np.ascontiguousarray(
                beta[os_].reshape(MT, 128).T.astype(np.float32)
            ),
            "mut": np.ascontiguousarray(
                mu[os_].reshape(MT, 128).T.astype(np.float32)
            ),
            "mu2": np.ascontiguousarray(
                (mu[os_] * mu[os_]).reshape(MT, 128).T.astype(np.float32)
            ),
        }
        wbs = wb[os_]
        if n8:
            # w8[m, p, t, i, o] = sign(W)[cs*1024 + m*128+o, (2t+i)*128 + p]
            cm["w8"] = np.ascontiguousarray(
                wbs[:, :split]
                .reshape(MT, 128, T8, 2, 128)
                .transpose(0, 4, 2, 3, 1)
                .astype(f8np)
            )
        if K16:
            # wt[m, p, k, o] = sign(W)[cs*1024 + m*128+o, (n8+k)*128+p]
            cm["wt"] = np.ascontiguousarray(
                wbs[:, split:]
                .reshape(MT, 128, K16, 128)
                .transpose(0, 3, 2, 1)
                .astype(_hdt)
            )
        cmaps.append(cm)

    # per-batch-shard x tensors (shared by the _CP cores on each shard)
    bmaps = []
    for bs_i in range(_DP):
        bs = slice(bs_i * b_loc, (bs_i + 1) * b_loc)
        bm = {}
        if n8:
            # x8[p, n, t, i, b] = xq[b0 + n*CH + b, (2t+i)*128 + p]
            bm["x8"] = np.ascontiguousarray(
                x8h[bs].reshape(NCH, CH, T8, 2, 128).transpose(4, 0, 2, 3, 1)
            )
        if K16:
            # xt[p, n, k, b] = xq[b0 + n*CH + b, (n8+k)*128+p]
            bm["xt"] = np.ascontiguousarray(
                x16h[bs].reshape(NCH, CH, K16, 128).transpose(3, 0, 2, 1)
            )
        bmaps.append(bm)

    in_maps = []
    for c in range(n_cores):
        im = dict(cmaps[c // _DP])
        im.update(bmaps[c % _DP])
        in_maps.append(im)
    return in_maps


def _gather_out(results, b_total, out_dim, n_cores):
    b_loc = b_total // _DP
    mt_loc = out_dim // 128 // _CP
    out = np.empty((b_total, out_dim), dtype=np.float32)
    for c in range(n_cores):
        bs = slice((c % _DP) * b_loc, (c % _DP + 1) * b_loc)
        os_ = slice((c // _DP) * mt_loc * 128, (c // _DP + 1) * mt_loc * 128)
        oc = np.asarray(results[c]["out"]).reshape(mt_loc, 128, b_loc)
        out[bs, os_] = (
            oc.transpose(2, 0, 1).reshape(b_loc, mt_loc * 128).astype(np.float32)
        )
    return out


def kernel(x, weight, gamma, beta):
    from concourse.bass_utils import run_bass_kernel_spmd

    x = np.asarray(x)
    weight = np.asarray(weight)
    gamma = np.asarray(gamma)
    beta = np.asarray(beta)

    n_cores = 8
    b_total, in_dim = x.shape
    out_dim = weight.shape[0]

    nc = _get_nc((n_cores, b_total, in_dim, out_dim))
    in_maps = _prepare_in_maps(x, weight, gamma, beta, n_cores)
    res = run_bass_kernel_spmd(nc, in_maps, list(range(n_cores)))
    return _gather_out(res.results, b_total, out_dim, n_cores)
